# revision 4
# baseline (speedup 1.0000x reference)
"""Trainium2 Bass kernel for nn_Decoder_81063212745440.

Pointer-network-style decoder: 4 sequential decode steps over a 4096-token
document. Each step: LSTM cell -> per-side (start/end) expert mixture with
maxout + HMN head -> per-position logits -> argmax feeds the next step.

Distribution: document dim m=4096 sharded across 8 cores (512 rows each).
Controller state is replicated. Per step each core computes its 512 local
logits; an AllGather of per-core (max, M-idx) pairs lets every core compute
the global argmax; the selected U rows are fetched by dynamic-offset DMA
from a DRAM copy of U.T.

Key structure (v2):
- z = U_loc @ We[:, :2d].T + be is step-invariant, precomputed once (f32r).
  Per-step work is the rank-1 v = We[:, 2d:] @ r bias + maxout + HMN.
- The per-step bias+maxout runs as fused scalar_tensor_tensor chains:
  acc_pt = (zU_pt + v_pt) max acc_{pt-1} -- one DVE op per pool slice.
- All per-step matvecs (LSTM, r/gate) run in bf16 (fp32 PE mode is ~2.7x
  slower); W2's b2 bias is folded into the PE accumulation via a rank-1
  ones matmul so PSUM drains are plain scalar copies.
- Warm-up AllGather is the first gpsimd instruction and has no consumers,
  so the NEFF-level collective barrier overlaps the preamble instead of
  stalling it.
- argmax encoding: enc = is_ge(logit, max) * (M - idx); max enc over all
  positions/cores = first global argmax. Global selection runs on the
  (otherwise idle) gpsimd engine so it is not stuck behind vector's FIFO.
"""

import ml_dtypes
import numpy as np

import concourse.bacc as bacc
import concourse.bass as bass
import concourse.mybir as mybir
import concourse.tile as tile
from concourse import bass_utils

D = 128          # hidden dim d
P = 8            # maxout pool width
K = 2            # experts
STEPS = 4
M = 4096         # document length
NCORES = 8
MLOC = M // NCORES   # 512 rows per core
F32 = mybir.dt.float32
BF16 = mybir.dt.bfloat16
F32R = mybir.dt.float32r
I32 = mybir.dt.int32
X = mybir.AxisListType.X
ALU = mybir.AluOpType
ACTF = mybir.ActivationFunctionType
ET = mybir.EngineType
SIDES = ("s", "e")

_CACHE = {}


def _build():
    """Build the SPMD Bass program (identical on all cores; data differs)."""
    nc = bacc.Bacc("TRN2", target_bir_lowering=False, debug=False,
                   num_devices=NCORES)

    # ---- I/O declarations ----------------------------------------------
    inp = {}

    def din(name, shape, dt=F32):
        inp[name] = nc.dram_tensor(name, list(shape), dt, kind="ExternalInput")
        return inp[name]

    din("uTdB", (128, 2 * M), BF16)    # U.T packed: [p, j*M+m] = U[m, j*128+p]
    din("ulocT", (128, 2 * MLOC), F32R)  # per-core U slice (f32, precompute)
    din("wihT", (128, 4 * 512), BF16)  # Wih.T k-tiles (gate order i,f,o,g)
    din("whhT", (128, 512), BF16)      # Whh.T
    din("bihhR", (1, 512), BF16)       # bih + bhh row
    din("iotaM", (128, 4))             # enc idx: [p,mt] = M - (c*512+mt*128+p)
    din("onesB", (1, 512), BF16)       # ones row (bf16 matmul helper)
    din("onesF", (1, 128))             # ones row (f32 bcast helper)
    din("ident", (128, 128))           # identity (PE partition transpose)
    din("zeroB", (128, 1), BF16)       # zero column (initial h)
    for s in SIDES:
        din(f"weT_{s}", (128, K * 2 * P * 128), F32R)
        din(f"weR_{s}", (128, K * P * 128), BF16)  # v matvec lhsT
        din(f"beC_{s}", (128, K * P))             # be cols per (expert, ptile)
        din(f"w2T_{s}", (128, P * 128), BF16)     # W2.T lhsT tiles
        din(f"b2R_{s}", (1, P * 128), BF16)       # b2 row (PE bias fold)
        din(f"w3T_{s}", (128, 2 * P), BF16)       # W3.T k-tiles
        din(f"b3F_{s}", (128, 4 * P))             # b3 bcast: [p, mt*8+pp]=b3[pp]
        din(f"wrgT_{s}", (128, 5 * 130), BF16)    # [Wr | Wg].T rhs k-tiles
        din(f"brgR_{s}", (1, 130), BF16)          # [br | bg] row

    out_d = {s: nc.dram_tensor(f"out_{s}", [STEPS, MLOC], F32,
                               kind="ExternalOutput") for s in SIDES}

    rg = [list(range(NCORES))]
    uTj = inp["uTdB"].rearrange("p (j c) -> p j c", j=2)

    with (
        tile.TileContext(nc) as tc,
        tc.tile_pool(name="consts", bufs=1) as constp,
        tc.tile_pool(name="dramw", bufs=1, space="DRAM") as dramw,
    ):
        # ---- warm-up AllGather: FIRST gpsimd instruction, no consumers.
        # Absorbs the NEFF collective barrier + inter-core start skew so
        # the step-0 AllGather doesn't pay it. Input is uninitialized
        # internal DRAM (result ignored).
        wag_in = dramw.tile([1, 8], F32, tag="wag_in")
        wag_out = dramw.tile([8, 8], F32, tag="wag_out")
        nc.gpsimd.collective_compute(
            "AllGather", ALU.bypass, replica_groups=rg,
            ins=[wag_in.opt()], outs=[wag_out.opt()])

        # ---- persistent SBUF constants ---------------------------------
        sb = {}
        _dmai = [0]

        def _ldconst(key):
            t = constp.tile(list(inp[key].shape), inp[key].dtype,
                            tag=f"sb_{key}", name=f"sb_{key}")
            eng = (nc.sync, nc.scalar)[_dmai[0] % 2]
            _dmai[0] += 1
            eng.dma_start(t[:], inp[key][:])
            sb[key] = t

        # zUb[side]: precomputed U_loc @ WeU.T + be, [e, m] layout.
        # col block (expert*P + ptile)*MLOC holds tile [128(d), 512(m)].
        zUb = {s: constp.tile([128, K * P * MLOC], BF16, tag=f"zUb_{s}",
                              name=f"zUb_{s}")
               for s in SIDES}

        # ---- precompute: the step-invariant expert GEMM ----------------
        with (
            tc.tile_pool(name="prew", bufs=2) as prew,
            tc.tile_pool(name="prepsum", bufs=4, space="PSUM") as prepsum,
        ):
            uloc = prew.tile([128, 2 * MLOC], F32R, tag="ulocT")
            nc.sync.dma_start(uloc[:], inp["ulocT"][:])
            wets = {}
            for i, s in enumerate(SIDES):
                wets[s] = prew.tile([128, K * 2 * P * 128], F32R, tag="weT",
                                    name=f"weT_{s}")
                (nc.sync if i == 0 else nc.scalar).dma_start(
                    wets[s][:], inp[f"weT_{s}"][:])
            for name in ("beC_s", "beC_e", "wihT", "whhT", "bihhR", "iotaM",
                         "onesB", "onesF", "ident", "zeroB"):
                _ldconst(name)
            for s in SIDES:
                for name in ("weR", "w2T", "b2R", "w3T", "b3F",
                             "wrgT", "brgR"):
                    _ldconst(f"{name}_{s}")
            for s in SIDES:
                wet = wets[s]
                for e in range(K):
                    for pt in range(P):
                        ps = prepsum.tile([128, MLOC], F32, tag="zps")
                        for kf in range(2):
                            lcol = ((e * 2 + kf) * P + pt) * 128
                            nc.tensor.matmul(
                                ps[:],
                                wet[:, lcol:lcol + 128],
                                uloc[:, kf * MLOC:(kf + 1) * MLOC],
                                start=(kf == 0), stop=(kf == 1),
                            )
                        blk = (e * P + pt) * MLOC
                        # fold static bias be while copying PSUM -> SBUF
                        dst = zUb[s][:, blk:blk + MLOC]
                        bcol = sb[f"beC_{s}"][:, e * P + pt:e * P + pt + 1]
                        if pt % 2 == 0:
                            nc.vector.tensor_scalar(dst, ps[:], bcol, None,
                                                    ALU.add)
                        else:
                            nc.scalar.activation(dst, ps[:], ACTF.Identity,
                                                 bias=bcol)

        oneB = sb["onesB"][0:1, 0:1]         # [1,1] == 1.0 (bf16)
        onesF = sb["onesF"][0:1, 0:128]      # [1,128] ones (f32)

        # ---- per-step pipeline -----------------------------------------
        with (
            tc.tile_pool(name="ctx", bufs=2) as ctxp,
            tc.tile_pool(name="hc", bufs=2) as hcp,
            tc.tile_pool(name="rows", bufs=4) as rowp,
            tc.tile_pool(name="chain", bufs=3) as chainp,  # bf16 chain tiles
            tc.tile_pool(name="sc", bufs=2) as scp,        # bf16 scratch
            tc.tile_pool(name="mx", bufs=2) as mxp,
            tc.tile_pool(name="lg", bufs=2) as lgp,
            tc.tile_pool(name="am", bufs=3) as amp,
            tc.tile_pool(name="psg", bufs=1, space="PSUM") as psg,
            tc.tile_pool(name="psr", bufs=2, space="PSUM") as psr,
            tc.tile_pool(name="psw", bufs=2, space="PSUM") as psw,
            tc.tile_pool(name="pss", bufs=3, space="PSUM") as pss,
            tc.tile_pool(name="dramp", bufs=2, space="DRAM") as dramp,
        ):
            h_col = sb["zeroB"][:]
            c_row = None
            # static gather offsets for step 0: si=0, ei=M-1
            us_cols = uTj[:, :, 0:1]
            ue_cols = uTj[:, :, M - 1:M]

            # rg PSUM tiles for step 0, bias pre-accumulated
            rg_ps = {}
            for s in SIDES:
                rg_ps[s] = psr.tile([1, 130], F32, tag="rg", name=f"rg0{s}")
                nc.tensor.matmul(rg_ps[s][:], oneB, sb[f"brgR_{s}"][:],
                                 start=True, stop=False)

            for t in range(STEPS):
                last = t == STEPS - 1
                # ---- gather us/ue from DRAM into a ctx tile ------------
                ctx = ctxp.tile([128, 4], BF16, tag="ctx")
                nc.sync.dma_start(ctx[:, 0:2], us_cols)
                nc.sync.dma_start(ctx[:, 2:4], ue_cols)

                # ---- LSTM cell (row layout) ----------------------------
                g_ps = psg.tile([1, 512], F32, tag="g")
                nc.tensor.matmul(g_ps[:], h_col, sb["whhT"][:],
                                 start=True, stop=False)
                nc.tensor.matmul(g_ps[:], oneB, sb["bihhR"][:],
                                 start=False, stop=False)
                for kx in range(4):
                    nc.tensor.matmul(
                        g_ps[:], ctx[:, kx:kx + 1],
                        sb["wihT"][:, kx * 512:(kx + 1) * 512],
                        start=False, stop=(kx == 3))
                    # rg matvec shares the ctx stationary (kc = 1+kx)
                    for s in SIDES:
                        kc = 1 + kx
                        nc.tensor.matmul(
                            rg_ps[s][:], ctx[:, kx:kx + 1],
                            sb[f"wrgT_{s}"][:, kc * 130:(kc + 1) * 130],
                            start=False, stop=False)

                # gates packed [i, f, o, g]: one sigmoid over 384 cols
                sig3 = rowp.tile([1, 384], F32, tag="sig3")
                tanh_g = rowp.tile([1, 128], F32, tag="tanh_g")
                nc.scalar.activation(sig3[:], g_ps[0:1, 0:384], ACTF.Sigmoid)
                nc.scalar.activation(tanh_g[:], g_ps[0:1, 384:512], ACTF.Tanh)

                ig = rowp.tile([1, 128], F32, tag="ig")
                c_new = hcp.tile([1, 128], F32, tag="c")
                nc.vector.tensor_tensor(ig[:], sig3[0:1, 0:128], tanh_g[:],
                                        ALU.mult)
                if c_row is None:
                    nc.vector.tensor_copy(c_new[:], ig[:])
                else:
                    fc = rowp.tile([1, 128], F32, tag="fc")
                    nc.vector.tensor_tensor(fc[:], sig3[0:1, 128:256], c_row,
                                            ALU.mult)
                    nc.vector.tensor_tensor(c_new[:], fc[:], ig[:], ALU.add)
                tanh_c = rowp.tile([1, 128], F32, tag="tanh_c")
                nc.scalar.activation(tanh_c[:], c_new[:], ACTF.Tanh)
                h_row = rowp.tile([1, 128], BF16, tag="h_row")
                nc.vector.tensor_tensor(h_row[:], sig3[0:1, 256:384],
                                        tanh_c[:], ALU.mult)
                c_row = c_new[:]

                # h transpose: [1,128] -> [128,1] via PE
                ht_ps = pss.tile([128, 2], F32, tag="pss")
                nc.tensor.matmul(ht_ps[:], h_row[:], sb["onesB"][0:1, 0:2],
                                 start=True, stop=True)
                h_new = hcp.tile([128, 1], BF16, tag="h_col")
                nc.vector.tensor_copy(h_new[:], ht_ps[:, 0:1])
                h_col = h_new[:]

                # finish rg with the h stationary (shared ldweights)
                for s in SIDES:
                    nc.tensor.matmul(rg_ps[s][:], h_col,
                                     sb[f"wrgT_{s}"][:, 0:130],
                                     start=False, stop=True)

                # ---- per-side r, gate, v -------------------------------
                r_col = {}
                gcol = {}
                vb = {}
                for s in SIDES:
                    r_row = rowp.tile([1, 128], BF16, tag=f"r_row{s}")
                    nc.scalar.activation(r_row[:], rg_ps[s][0:1, 0:128],
                                         ACTF.Tanh)
                    # gate (K=2): g0 = sigmoid(a0 - a1), g1 = 1 - g0
                    gg = rowp.tile([1, 2], F32, tag=f"gg{s}")
                    nc.vector.tensor_copy(gg[:], rg_ps[s][0:1, 128:130])
                    gd = rowp.tile([1, 1], F32, tag=f"gd{s}")
                    nc.vector.tensor_tensor(gd[:], gg[0:1, 0:1], gg[0:1, 1:2],
                                            ALU.subtract)
                    g01 = rowp.tile([1, 2], F32, tag=f"g01{s}")
                    nc.scalar.activation(g01[0:1, 0:1], gd[:], ACTF.Sigmoid)
                    nc.vector.tensor_scalar(g01[0:1, 1:2], g01[0:1, 0:1],
                                            -1.0, 1.0, ALU.mult, ALU.add)
                    # transpose r; broadcast gates to columns
                    rt_ps = pss.tile([128, 2], F32, tag="pss")
                    nc.tensor.matmul(rt_ps[:], r_row[:], sb["onesB"][0:1, 0:2],
                                     start=True, stop=True)
                    rc = rowp.tile([128, 1], BF16, tag=f"r_col{s}")
                    nc.vector.tensor_copy(rc[:], rt_ps[:, 0:1])
                    r_col[s] = rc
                    gb_ps = pss.tile([128, 2], F32, tag="pss")
                    nc.tensor.matmul(gb_ps[:], onesF, g01[:],
                                     start=True, stop=True)
                    gc = rowp.tile([128, 2], F32, tag=f"gcol{s}")
                    nc.vector.tensor_copy(gc[:], gb_ps[:])
                    gcol[s] = gc
                    # v matvec: 8 column matmuls per expert
                    for e in range(K):
                        v_ps = pss.tile([128, P], F32, tag="pss")
                        for pt in range(P):
                            lcol = (e * P + pt) * 128
                            nc.tensor.matmul(
                                v_ps[:, pt:pt + 1],
                                sb[f"weR_{s}"][:, lcol:lcol + 128],
                                r_col[s][:], start=True, stop=True)
                        vbt = rowp.tile([128, P], F32, tag=f"vb{s}{e}")
                        nc.vector.tensor_copy(vbt[:], v_ps[:])
                        vb[(s, e)] = vbt

                # ---- expert bias+maxout: fused STT chains --------------
                # acc_pt = (zU_pt + v_pt) max acc_{pt-1}; two sides' chains
                # interleaved so the vector FIFO never stalls.
                mx = {}
                for s in SIDES:
                    acc = {}
                    for pt in range(P):
                        for e in range(K):
                            blk = (e * P + pt) * MLOC
                            src = zUb[s][:, blk:blk + MLOC]
                            vcol = vb[(s, e)][:, pt:pt + 1]
                            nt = chainp.tile([128, MLOC], BF16,
                                             tag=f"ch{s}{e}",
                                             name=f"ch{t}{s}{e}{pt}")
                            if pt == 0:
                                nc.vector.tensor_scalar(nt[:], src, vcol,
                                                        None, ALU.add)
                            else:
                                nc.vector.scalar_tensor_tensor(
                                    nt[:], src, vcol, acc[e][:],
                                    ALU.add, ALU.max)
                            acc[e] = nt
                        mx[s] = acc

                # ---- mixture: m1 = g0*mx0 + g1*mx1 ---------------------
                m1 = {}
                for s in SIDES:
                    tm = scp.tile([128, MLOC], BF16, tag=f"tm{s}")
                    nc.vector.tensor_scalar(tm[:], mx[s][0][:],
                                            gcol[s][:, 0:1], None, ALU.mult)
                    m1t = mxp.tile([128, MLOC], BF16, tag=f"m1{s}")
                    nc.vector.scalar_tensor_tensor(
                        m1t[:], mx[s][1][:], gcol[s][:, 1:2], tm[:],
                        ALU.mult, ALU.add)
                    m1[s] = m1t

                # ---- HMN W2 + maxout, logits, local argmax per side ----
                agin = amp.tile([1, 8], F32, tag="agin")
                if not last:
                    nc.gpsimd.memset(agin[:], 0.0)
                for si_, s in enumerate(SIDES):
                    # W2: bias folded into PE accumulation; drains are
                    # plain scalar Identity copies into w2ball.
                    w2ball = scp.tile([128, P * MLOC], BF16, tag="w2ball",
                                      name=f"w2ball{t}{si_}")
                    for pt in range(P):
                        ps = psw.tile([128, MLOC], F32, tag="w2ps")
                        nc.tensor.matmul(
                            ps[:], sb[f"b2R_{s}"][0:1, pt * 128:(pt + 1) * 128],
                            sb["onesB"][0:1, 0:MLOC], start=True, stop=False)
                        nc.tensor.matmul(
                            ps[:], sb[f"w2T_{s}"][:, pt * 128:(pt + 1) * 128],
                            m1[s][:], start=False, stop=True)
                        nc.scalar.activation(
                            w2ball[:, pt * MLOC:(pt + 1) * MLOC], ps[:],
                            ACTF.Identity)
                    qt1 = scp.tile([128, 4 * MLOC], BF16, tag="qt1",
                                   name=f"qt1{t}{si_}")
                    nc.vector.tensor_tensor(qt1[:], w2ball[:, 0:4 * MLOC],
                                            w2ball[:, 4 * MLOC:8 * MLOC],
                                            ALU.max)
                    qt2 = scp.tile([128, 2 * MLOC], BF16, tag="qt2",
                                   name=f"qt2{t}{si_}")
                    nc.vector.tensor_tensor(qt2[:], qt1[:, 0:2 * MLOC],
                                            qt1[:, 2 * MLOC:4 * MLOC],
                                            ALU.max)
                    m2 = mxp.tile([128, MLOC], BF16, tag=f"m2{s}")
                    nc.vector.tensor_tensor(m2[:], qt2[:, 0:MLOC],
                                            qt2[:, MLOC:2 * MLOC], ALU.max)

                    # logits: pool dim on the free axis
                    l_ps = pss.tile([128, 4 * P], F32, tag="pss")
                    for mt in range(4):
                        nc.tensor.matmul(
                            l_ps[:, mt * P:(mt + 1) * P],
                            m1[s][:, mt * 128:(mt + 1) * 128],
                            sb[f"w3T_{s}"][:, 0:P], start=True, stop=False)
                        nc.tensor.matmul(
                            l_ps[:, mt * P:(mt + 1) * P],
                            m2[:, mt * 128:(mt + 1) * 128],
                            sb[f"w3T_{s}"][:, P:2 * P], start=False, stop=True)
                    lgb = lgp.tile([128, 4 * P], F32, tag="lgb")
                    nc.vector.tensor_tensor(lgb[:], l_ps[:], sb[f"b3F_{s}"][:],
                                            ALU.add)
                    # max over pool pp (innermost, stride 1): [128, 4]
                    lgc = lgp.tile([128, 4], F32, tag="lgc")
                    nc.vector.tensor_reduce(
                        lgc[:], lgb[:].rearrange("p (mt pp) -> p mt pp", pp=P),
                        axis=X, op=ALU.max)
                    nc.sync.dma_start(out_d[s][t:t + 1, :], lgc[:])

                    # ---- local (max, enc=M-idx) over [128, 4] ----------
                    if not last:
                        cmax = amp.tile([128, 1], F32, tag="cmax")
                        nc.vector.tensor_reduce(cmax[:], lgc[:], axis=X,
                                                op=ALU.max)
                        rmax_ps = pss.tile([1, 128], F32, tag="pss")
                        nc.tensor.matmul(rmax_ps[:], cmax[:], sb["ident"][:],
                                         start=True, stop=True)
                        nc.vector.tensor_reduce(
                            agin[0:1, 2 * si_:2 * si_ + 1], rmax_ps[:],
                            axis=X, op=ALU.max)
                        mb_ps = pss.tile([128, 1], F32, tag="pss")
                        nc.tensor.matmul(mb_ps[:], onesF,
                                         agin[0:1, 2 * si_:2 * si_ + 1],
                                         start=True, stop=True)
                        lmaxb = amp.tile([128, 1], F32, tag="lmaxb")
                        nc.vector.tensor_copy(lmaxb[:], mb_ps[:])
                        enc = amp.tile([128, 4], F32, tag="enc")
                        nc.vector.scalar_tensor_tensor(
                            enc[:], lgc[:], lmaxb[:, 0:1], sb["iotaM"][:],
                            ALU.is_ge, ALU.mult)
                        cenc = amp.tile([128, 1], F32, tag="cenc")
                        nc.vector.tensor_reduce(cenc[:], enc[:], axis=X,
                                                op=ALU.max)
                        renc_ps = pss.tile([1, 128], F32, tag="pss")
                        nc.tensor.matmul(renc_ps[:], cenc[:], sb["ident"][:],
                                         start=True, stop=True)
                        nc.vector.tensor_reduce(
                            agin[0:1, 2 * si_ + 1:2 * si_ + 2], renc_ps[:],
                            axis=X, op=ALU.max)

                # ---- AllGather of (max, enc) pairs; global argmax ------
                if not last:
                    ag_in = dramp.tile([1, 8], F32, tag="ag_in")
                    ag_out = dramp.tile([8, 8], F32, tag="ag_out")
                    nc.sync.dma_start(ag_in[:], agin[:])
                    nc.gpsimd.collective_compute(
                        "AllGather", ALU.bypass, replica_groups=rg,
                        ins=[ag_in.opt()], outs=[ag_out.opt()])
                    # agb cols: j*8 + rank, j in (max_s, enc_s, max_e, enc_e)
                    agb = amp.tile([1, 32], F32, tag="agb")
                    nc.sync.dma_start(
                        agb[:].rearrange("a (j r) -> a j r", r=8),
                        ag_out[:, 0:4].transpose([1, 0]))
                    # global argmax (gpsimd can't do free-axis reduce)
                    idxw = amp.tile([1, 2], F32, tag="idxw")
                    for si_ in range(2):
                        cols = agb[0:1, 16 * si_:16 * si_ + 8]
                        encs = agb[0:1, 16 * si_ + 8:16 * si_ + 16]
                        gmax = amp.tile([1, 1], F32, tag=f"gmax{si_}")
                        nc.vector.tensor_reduce(gmax[:], cols, axis=X,
                                                op=ALU.max)
                        gsel = amp.tile([1, 8], F32, tag=f"gsel{si_}")
                        nc.vector.scalar_tensor_tensor(
                            gsel[:], cols, gmax[0:1, 0:1], encs,
                            ALU.is_ge, ALU.mult)
                        genc = amp.tile([1, 1], F32, tag=f"genc{si_}")
                        nc.vector.tensor_reduce(genc[:], gsel[:], axis=X,
                                                op=ALU.max)
                        # idx = M - enc
                        nc.vector.tensor_scalar(
                            idxw[0:1, si_:si_ + 1], genc[:], -1.0, float(M),
                            ALU.mult, ALU.add)
                    idx2i = amp.tile([1, 2], I32, tag="idx2i")
                    nc.vector.tensor_copy(idx2i[:], idxw[:])
                    si_v = nc.values_load(idx2i[0:1, 0:1], engines=(ET.SP,),
                                          min_val=0, max_val=M - 1,
                                          skip_runtime_bounds_check=True)
                    ei_v = nc.values_load(idx2i[0:1, 1:2], engines=(ET.SP,),
                                          min_val=0, max_val=M - 1,
                                          skip_runtime_bounds_check=True)
                    us_cols = uTj[:, :, bass.ds(si_v, 1)]
                    ue_cols = uTj[:, :, bass.ds(ei_v, 1)]

                    # pre-accumulate next step's rg bias while AG settles
                    rg_next = {}
                    for s in SIDES:
                        rg_next[s] = psr.tile([1, 130], F32, tag="rg",
                                              name=f"rg{t + 1}{s}")
                        nc.tensor.matmul(rg_next[s][:], oneB,
                                         sb[f"brgR_{s}"][:],
                                         start=True, stop=False)
                    rg_ps = rg_next

    nc.compile()
    return nc


def _pack_inputs(full):
    """Split/transform full inputs into 8 per-core input maps."""
    U = np.ascontiguousarray(np.asarray(full["U"], np.float32)[0])  # (M, 2D)
    d = D
    bf = ml_dtypes.bfloat16
    common = {}
    # uTdB: [p, j*M + m] = U[m, j*128 + p]
    uTd = np.empty((128, 2 * M), np.float32)
    for j in range(2):
        uTd[:, j * M:(j + 1) * M] = U[:, j * 128:(j + 1) * 128].T
    common["uTdB"] = uTd.astype(bf)
    perm = np.concatenate([np.arange(0, 256), np.arange(384, 512),
                           np.arange(256, 384)])     # [i, f, o, g]
    Wih = np.asarray(full["lstm_Wih"], np.float32)[perm]    # (512, 512)
    WihT = Wih.T                                      # [x, g]
    wihT = np.empty((128, 4 * 512), np.float32)
    for kx in range(4):
        wihT[:, kx * 512:(kx + 1) * 512] = WihT[kx * 128:(kx + 1) * 128, :]
    common["wihT"] = wihT.astype(bf)
    common["whhT"] = np.ascontiguousarray(
        np.asarray(full["lstm_Whh"], np.float32)[perm].T).astype(bf)
    common["bihhR"] = ((np.asarray(full["lstm_bih"], np.float32)
                        + np.asarray(full["lstm_bhh"], np.float32))[perm]
                       [None, :]).astype(bf)
    common["onesB"] = np.ones((1, 512), bf)
    common["onesF"] = np.ones((1, 128), np.float32)
    common["ident"] = np.eye(128, dtype=np.float32)
    common["zeroB"] = np.zeros((128, 1), bf)

    for s in SIDES:
        We = np.asarray(full[f"We_{s}"], np.float32)      # (K, P*D, 3D)
        be = np.asarray(full[f"be_{s}"], np.float32)      # (K, P*D)
        weT = np.empty((128, K * 2 * P * 128), np.float32)
        weR = np.empty((128, K * P * 128), np.float32)
        beC = np.empty((128, K * P), np.float32)
        for e in range(K):
            for kf in range(2):
                for pt in range(P):
                    col = ((e * 2 + kf) * P + pt) * 128
                    # lhsT[f, ec] = We[e, pt*128+ec, kf*128+f]
                    weT[:, col:col + 128] = We[e, pt * 128:(pt + 1) * 128,
                                               kf * 128:(kf + 1) * 128].T
            for pt in range(P):
                col = (e * P + pt) * 128
                weR[:, col:col + 128] = We[e, pt * 128:(pt + 1) * 128,
                                           2 * d:3 * d].T
                beC[:, e * P + pt] = be[e, pt * 128:(pt + 1) * 128]
        common[f"weT_{s}"] = weT
        common[f"weR_{s}"] = weR.astype(bf)
        common[f"beC_{s}"] = beC

        W2 = np.asarray(full[f"W2_{s}"], np.float32)      # (P*D, D)
        w2T = np.empty((128, P * 128), np.float32)
        for pt in range(P):
            w2T[:, pt * 128:(pt + 1) * 128] = W2[pt * 128:(pt + 1) * 128, :].T
        common[f"w2T_{s}"] = w2T.astype(bf)
        common[f"b2R_{s}"] = np.asarray(full[f"b2_{s}"],
                                        np.float32)[None, :].astype(bf)

        W3 = np.asarray(full[f"W3_{s}"], np.float32)      # (P, 2D)
        w3T = np.empty((128, 2 * P), np.float32)
        for kf in range(2):
            w3T[:, kf * P:(kf + 1) * P] = W3[:, kf * 128:(kf + 1) * 128].T
        common[f"w3T_{s}"] = w3T.astype(bf)
        b3 = np.asarray(full[f"b3_{s}"], np.float32)
        common[f"b3F_{s}"] = np.broadcast_to(
            np.tile(b3, 4)[None, :], (128, 4 * P)).copy()

        WrT = np.asarray(full[f"Wr_{s}"], np.float32).T   # [ctx, i]
        WgT = np.asarray(full[f"Wg_{s}"], np.float32).T   # [ctx, j]
        wrgT = np.empty((128, 5 * 130), np.float32)
        for kc in range(5):
            wrgT[:, kc * 130:kc * 130 + 128] = WrT[kc * 128:(kc + 1) * 128, :]
            wrgT[:, kc * 130 + 128:(kc + 1) * 130] = \
                WgT[kc * 128:(kc + 1) * 128, :]
        common[f"wrgT_{s}"] = wrgT.astype(bf)
        common[f"brgR_{s}"] = np.concatenate(
            [np.asarray(full[f"br_{s}"], np.float32),
             np.asarray(full[f"bg_{s}"], np.float32)])[None, :].astype(bf)

    in_maps = []
    for c in range(NCORES):
        m = dict(common)
        io = np.empty((128, 4), np.float32)
        for mt in range(4):
            io[:, mt] = M - (c * MLOC + mt * 128 + np.arange(128))
        m["iotaM"] = io
        ulocT = np.empty((128, 2 * MLOC), np.float32)
        for j in range(2):
            ulocT[:, j * MLOC:(j + 1) * MLOC] = \
                U[c * MLOC:(c + 1) * MLOC, j * 128:(j + 1) * 128].T
        m["ulocT"] = ulocT
        in_maps.append(m)
    return in_maps


def kernel(**inputs):
    if "nc" not in _CACHE:
        _CACHE["nc"] = _build()
    nc = _CACHE["nc"]
    in_maps = _pack_inputs(inputs)
    res = bass_utils.run_bass_kernel_spmd(
        nc, in_maps, core_ids=list(range(NCORES)))
    starts = np.empty((1, STEPS, M), np.float32)
    ends = np.empty((1, STEPS, M), np.float32)
    for c in range(NCORES):
        for dst, key in ((starts, "out_s"), (ends, "out_e")):
            raw = res.results[c][key]                       # [4, 512] (p,mt)
            dst[0, :, c * MLOC:(c + 1) * MLOC] = (
                raw.reshape(STEPS, 128, 4).transpose(0, 2, 1)
                .reshape(STEPS, MLOC))
    return starts, ends


# revision 6
# speedup vs baseline: 1.2288x; 1.2288x over previous
"""Trainium2 Bass kernel for nn_Decoder_81063212745440.

Pointer-network-style decoder: 4 sequential decode steps over a 4096-token
document. Each step: LSTM cell -> per-side (start/end) expert mixture with
maxout + HMN head -> per-position logits -> argmax feeds the next step.

Distribution: document dim m=4096 sharded across 8 cores (512 rows each).
Controller state is replicated. Per step each core computes its 512 local
logits; an AllGather of per-core (max, M-idx) pairs lets every core compute
the global argmax; the selected U rows are fetched by dynamic-offset DMA
from a DRAM copy of U.T.

Key structure (v3):
- z = U_loc @ We[:, :2d].T + be is step-invariant, precomputed once (f32r).
  Per-step work is the rank-1 v = We[:, 2d:] @ r bias + maxout + HMN.
- zUb blocks are laid out (pt, e)-interleaved so the two experts' maxout
  trees coalesce into one 3-level tree of wide TTs per side (measured DVE
  modes: TS bf16 ~400ns, TT bf16 2x; fused STT runs 1x -- not used).
- All per-step matvecs (LSTM, r/gate) run in bf16; W2 drains fold b2 on
  the scalar engine; b3 folds into the logits PSUM accumulation via tiny
  rank-1 matmuls.
- Warm-up AllGather is the first gpsimd instruction and has no consumers,
  so the NEFF-level collective barrier overlaps the preamble.
- argmax encoding: enc = is_ge(logit, max) * (M - idx); max enc over all
  positions/cores = first global argmax.
"""

import ml_dtypes
import numpy as np

import concourse.bacc as bacc
import concourse.bass as bass
import concourse.mybir as mybir
import concourse.tile as tile
from concourse import bass_utils

D = 128          # hidden dim d
P = 8            # maxout pool width
K = 2            # experts
STEPS = 4
M = 4096         # document length
NCORES = 8
MLOC = M // NCORES   # 512 rows per core
F32 = mybir.dt.float32
BF16 = mybir.dt.bfloat16
F32R = mybir.dt.float32r
I32 = mybir.dt.int32
X = mybir.AxisListType.X
ALU = mybir.AluOpType
ACTF = mybir.ActivationFunctionType
ET = mybir.EngineType
SIDES = ("s", "e")

_CACHE = {}


def _build():
    """Build the SPMD Bass program (identical on all cores; data differs)."""
    nc = bacc.Bacc("TRN2", target_bir_lowering=False, debug=False,
                   num_devices=NCORES)

    # ---- I/O declarations ----------------------------------------------
    inp = {}

    def din(name, shape, dt=F32):
        inp[name] = nc.dram_tensor(name, list(shape), dt, kind="ExternalInput")
        return inp[name]

    din("uTdB", (128, 2 * M), BF16)    # U.T packed: [p, j*M+m] = U[m, j*128+p]
    din("ulocT", (128, 2 * MLOC), F32R)  # per-core U slice (f32, precompute)
    din("wihT", (128, 4 * 512), BF16)  # Wih.T k-tiles (gate order i,f,o,g)
    din("whhT", (128, 512), BF16)      # Whh.T
    din("bihhR", (1, 512), BF16)       # bih + bhh row
    din("iotaM", (128, 4))             # enc idx: [p,mt] = M - (c*512+mt*128+p)
    din("onesB", (1, 512), BF16)       # ones row (bf16 matmul helper)
    din("onesF", (1, 128))             # ones row (f32 bcast helper)
    din("ident", (128, 128))           # identity (PE partition transpose)
    din("zeroB", (128, 1), BF16)       # zero column (initial h)
    for s in SIDES:
        din(f"weT_{s}", (128, K * 2 * P * 128), F32R)
        din(f"weR_{s}", (128, K * P * 128), BF16)  # v matvec lhsT, (pt,e) order
        din(f"beC_{s}", (128, K * P))             # be cols, col = pt*K+e
        din(f"w2T_{s}", (128, P * 128), BF16)     # W2.T lhsT tiles
        din(f"b2C_{s}", (128, P))                 # b2 cols per ptile
        din(f"w3T_{s}", (128, 2 * P), BF16)       # W3.T k-tiles
        din(f"b3R_{s}", (1, P))                   # b3 row (PE bias fold)
        din(f"wrgT_{s}", (128, 5 * 130), BF16)    # [Wr | Wg].T rhs k-tiles
        din(f"brgR_{s}", (1, 130), BF16)          # [br | bg] row

    out_d = {s: nc.dram_tensor(f"out_{s}", [STEPS, MLOC], F32,
                               kind="ExternalOutput") for s in SIDES}

    rg = [list(range(NCORES))]
    uTj = inp["uTdB"].rearrange("p (j c) -> p j c", j=2)

    with (
        tile.TileContext(nc) as tc,
        tc.tile_pool(name="consts", bufs=1) as constp,
        tc.tile_pool(name="dramw", bufs=1, space="DRAM") as dramw,
    ):
        # ---- warm-up AllGather: FIRST gpsimd instruction, no consumers.
        wag_in = dramw.tile([1, 8], F32, tag="wag_in")
        wag_out = dramw.tile([8, 8], F32, tag="wag_out")
        nc.gpsimd.collective_compute(
            "AllGather", ALU.bypass, replica_groups=rg,
            ins=[wag_in.opt()], outs=[wag_out.opt()])

        # ---- persistent SBUF constants ---------------------------------
        sb = {}
        _dmai = [0]

        def _ldconst(key):
            t = constp.tile(list(inp[key].shape), inp[key].dtype,
                            tag=f"sb_{key}", name=f"sb_{key}")
            eng = (nc.sync, nc.scalar)[_dmai[0] % 2]
            _dmai[0] += 1
            eng.dma_start(t[:], inp[key][:])
            sb[key] = t

        # zUb[side]: precomputed U_loc @ WeU.T + be.
        # col block (pt*K + e)*MLOC holds tile [128(d), 512(m)].
        zUb = {s: constp.tile([128, K * P * MLOC], BF16, tag=f"zUb_{s}",
                              name=f"zUb_{s}")
               for s in SIDES}

        # ---- precompute: the step-invariant expert GEMM ----------------
        with (
            tc.tile_pool(name="prew", bufs=2) as prew,
            tc.tile_pool(name="prepsum", bufs=4, space="PSUM") as prepsum,
        ):
            uloc = prew.tile([128, 2 * MLOC], F32R, tag="ulocT")
            nc.sync.dma_start(uloc[:], inp["ulocT"][:])
            wets = {}
            for i, s in enumerate(SIDES):
                wets[s] = prew.tile([128, K * 2 * P * 128], F32R, tag="weT",
                                    name=f"weT_{s}")
                (nc.sync if i == 0 else nc.scalar).dma_start(
                    wets[s][:], inp[f"weT_{s}"][:])
            for name in ("beC_s", "beC_e", "wihT", "whhT", "bihhR", "iotaM",
                         "onesB", "onesF", "ident", "zeroB"):
                _ldconst(name)
            for s in SIDES:
                for name in ("weR", "w2T", "b2C", "w3T", "b3R",
                             "wrgT", "brgR"):
                    _ldconst(f"{name}_{s}")
            for s in SIDES:
                wet = wets[s]
                for e in range(K):
                    for pt in range(P):
                        ps = prepsum.tile([128, MLOC], F32, tag="zps")
                        for kf in range(2):
                            lcol = ((e * 2 + kf) * P + pt) * 128
                            nc.tensor.matmul(
                                ps[:],
                                wet[:, lcol:lcol + 128],
                                uloc[:, kf * MLOC:(kf + 1) * MLOC],
                                start=(kf == 0), stop=(kf == 1),
                            )
                        blk = (pt * K + e) * MLOC
                        dst = zUb[s][:, blk:blk + MLOC]
                        bcol = sb[f"beC_{s}"][:, pt * K + e:pt * K + e + 1]
                        if pt % 2 == 0:
                            nc.vector.tensor_scalar(dst, ps[:], bcol, None,
                                                    ALU.add)
                        else:
                            nc.scalar.activation(dst, ps[:], ACTF.Identity,
                                                 bias=bcol)

        oneB = sb["onesB"][0:1, 0:1]         # [1,1] == 1.0 (bf16)
        onesF = sb["onesF"][0:1, 0:128]      # [1,128] ones (f32)

        # ---- per-step pipeline -----------------------------------------
        with (
            tc.tile_pool(name="ctx", bufs=2) as ctxp,
            tc.tile_pool(name="hc", bufs=2) as hcp,
            tc.tile_pool(name="rows", bufs=4) as rowp,
            tc.tile_pool(name="zb", bufs=1) as zbp,        # bf16 zball
            tc.tile_pool(name="sc", bufs=2) as scp,        # bf16 scratch
            tc.tile_pool(name="mx", bufs=2) as mxp,
            tc.tile_pool(name="lg", bufs=2) as lgp,
            tc.tile_pool(name="am", bufs=3) as amp,
            tc.tile_pool(name="psg", bufs=1, space="PSUM") as psg,
            tc.tile_pool(name="psr", bufs=2, space="PSUM") as psr,
            tc.tile_pool(name="psw", bufs=2, space="PSUM") as psw,
            tc.tile_pool(name="pss", bufs=3, space="PSUM") as pss,
            tc.tile_pool(name="dramp", bufs=2, space="DRAM") as dramp,
        ):
            h_col = sb["zeroB"][:]
            c_row = None
            # static gather offsets for step 0: si=0, ei=M-1
            us_cols = uTj[:, :, 0:1]
            ue_cols = uTj[:, :, M - 1:M]

            # rg PSUM tiles for step 0, bias pre-accumulated
            rg_ps = {}
            for s in SIDES:
                rg_ps[s] = psr.tile([1, 130], F32, tag="rg", name=f"rg0{s}")
                nc.tensor.matmul(rg_ps[s][:], oneB, sb[f"brgR_{s}"][:],
                                 start=True, stop=False)

            for t in range(STEPS):
                last = t == STEPS - 1
                # ---- gather us/ue from DRAM into a ctx tile ------------
                ctx = ctxp.tile([128, 4], BF16, tag="ctx")
                nc.sync.dma_start(ctx[:, 0:2], us_cols)
                (nc.scalar if t > 0 else nc.sync).dma_start(
                    ctx[:, 2:4], ue_cols)

                # ---- LSTM cell (row layout) ----------------------------
                g_ps = psg.tile([1, 512], F32, tag="g")
                nc.tensor.matmul(g_ps[:], h_col, sb["whhT"][:],
                                 start=True, stop=False)
                nc.tensor.matmul(g_ps[:], oneB, sb["bihhR"][:],
                                 start=False, stop=False)
                for kx in range(4):
                    nc.tensor.matmul(
                        g_ps[:], ctx[:, kx:kx + 1],
                        sb["wihT"][:, kx * 512:(kx + 1) * 512],
                        start=False, stop=(kx == 3))
                    # rg matvec shares the ctx stationary (kc = 1+kx)
                    for s in SIDES:
                        kc = 1 + kx
                        nc.tensor.matmul(
                            rg_ps[s][:], ctx[:, kx:kx + 1],
                            sb[f"wrgT_{s}"][:, kc * 130:(kc + 1) * 130],
                            start=False, stop=False)

                # gates packed [i, f, o, g]: one sigmoid over 384 cols
                sig3 = rowp.tile([1, 384], F32, tag="sig3")
                tanh_g = rowp.tile([1, 128], F32, tag="tanh_g")
                nc.scalar.activation(sig3[:], g_ps[0:1, 0:384], ACTF.Sigmoid)
                nc.scalar.activation(tanh_g[:], g_ps[0:1, 384:512], ACTF.Tanh)

                ig = rowp.tile([1, 128], F32, tag="ig")
                c_new = hcp.tile([1, 128], F32, tag="c")
                nc.vector.tensor_tensor(ig[:], sig3[0:1, 0:128], tanh_g[:],
                                        ALU.mult)
                if c_row is None:
                    nc.vector.tensor_copy(c_new[:], ig[:])
                else:
                    fc = rowp.tile([1, 128], F32, tag="fc")
                    nc.vector.tensor_tensor(fc[:], sig3[0:1, 128:256], c_row,
                                            ALU.mult)
                    nc.vector.tensor_tensor(c_new[:], fc[:], ig[:], ALU.add)
                tanh_c = rowp.tile([1, 128], F32, tag="tanh_c")
                nc.scalar.activation(tanh_c[:], c_new[:], ACTF.Tanh)
                h_row = rowp.tile([1, 128], BF16, tag="h_row")
                nc.vector.tensor_tensor(h_row[:], sig3[0:1, 256:384],
                                        tanh_c[:], ALU.mult)
                c_row = c_new[:]

                # h transpose: [1,128] -> [128,1] via PE
                ht_ps = pss.tile([128, 2], F32, tag="pss")
                nc.tensor.matmul(ht_ps[:], h_row[:], sb["onesB"][0:1, 0:2],
                                 start=True, stop=True)
                h_new = hcp.tile([128, 1], BF16, tag="h_col")
                nc.vector.tensor_copy(h_new[:], ht_ps[:, 0:1])
                h_col = h_new[:]

                # finish rg with the h stationary (shared ldweights)
                for s in SIDES:
                    nc.tensor.matmul(rg_ps[s][:], h_col,
                                     sb[f"wrgT_{s}"][:, 0:130],
                                     start=False, stop=True)

                # ---- per-side r, gate, v -------------------------------
                gcol = {}
                vb = {}
                for s in SIDES:
                    r_row = rowp.tile([1, 128], BF16, tag=f"r_row{s}")
                    nc.scalar.activation(r_row[:], rg_ps[s][0:1, 0:128],
                                         ACTF.Tanh)
                    # gate (K=2): g0 = sigmoid(a0 - a1), g1 = 1 - g0
                    gg = rowp.tile([1, 2], F32, tag=f"gg{s}")
                    nc.vector.tensor_copy(gg[:], rg_ps[s][0:1, 128:130])
                    gd = rowp.tile([1, 1], F32, tag=f"gd{s}")
                    nc.vector.tensor_tensor(gd[:], gg[0:1, 0:1], gg[0:1, 1:2],
                                            ALU.subtract)
                    g01 = rowp.tile([1, 2], F32, tag=f"g01{s}")
                    nc.scalar.activation(g01[0:1, 0:1], gd[:], ACTF.Sigmoid)
                    nc.vector.tensor_scalar(g01[0:1, 1:2], g01[0:1, 0:1],
                                            -1.0, 1.0, ALU.mult, ALU.add)
                    # transpose r; broadcast gates to columns
                    rt_ps = pss.tile([128, 2], F32, tag="pss")
                    nc.tensor.matmul(rt_ps[:], r_row[:], sb["onesB"][0:1, 0:2],
                                     start=True, stop=True)
                    rc = rowp.tile([128, 1], BF16, tag=f"r_col{s}")
                    nc.vector.tensor_copy(rc[:], rt_ps[:, 0:1])
                    gb_ps = pss.tile([128, 2], F32, tag="pss")
                    nc.tensor.matmul(gb_ps[:], onesF, g01[:],
                                     start=True, stop=True)
                    gc = rowp.tile([128, 2], F32, tag=f"gcol{s}")
                    nc.vector.tensor_copy(gc[:], gb_ps[:])
                    gcol[s] = gc
                    # v matvec: 16 column matmuls, col = pt*K+e
                    v_ps = pss.tile([128, K * P], F32, tag="pss")
                    for pt in range(P):
                        for e in range(K):
                            lcol = (pt * K + e) * 128
                            nc.tensor.matmul(
                                v_ps[:, pt * K + e:pt * K + e + 1],
                                sb[f"weR_{s}"][:, lcol:lcol + 128],
                                rc[:], start=True, stop=True)
                    vbt = rowp.tile([128, K * P], F32, tag=f"vb{s}")
                    nc.vector.tensor_copy(vbt[:], v_ps[:])
                    vb[s] = vbt

                # ---- expert bias + coalesced maxout tree ---------------
                mxw = {}
                for s in SIDES:
                    zball = zbp.tile([128, K * P * MLOC], BF16, tag=f"zb{s}",
                                     name=f"zb{t}{s}")
                    for b in range(K * P):
                        dst = zball[:, b * MLOC:(b + 1) * MLOC]
                        src = zUb[s][:, b * MLOC:(b + 1) * MLOC]
                        vcol = vb[s][:, b:b + 1]
                        nc.vector.tensor_scalar(dst, src, vcol, None, ALU.add)
                    ht1 = scp.tile([128, 8 * MLOC], BF16, tag="ht1",
                                   name=f"ht1{t}{s}")
                    nc.vector.tensor_tensor(ht1[:], zball[:, 0:8 * MLOC],
                                            zball[:, 8 * MLOC:16 * MLOC],
                                            ALU.max)
                    ht2 = scp.tile([128, 4 * MLOC], BF16, tag="ht2",
                                   name=f"ht2{t}{s}")
                    nc.vector.tensor_tensor(ht2[:], ht1[:, 0:4 * MLOC],
                                            ht1[:, 4 * MLOC:8 * MLOC],
                                            ALU.max)
                    mxe = mxp.tile([128, K * MLOC], BF16, tag=f"mx{s}")
                    nc.vector.tensor_tensor(mxe[:], ht2[:, 0:2 * MLOC],
                                            ht2[:, 2 * MLOC:4 * MLOC],
                                            ALU.max)
                    mxw[s] = mxe

                # ---- mixture: m1 = g0*mx0 + g1*mx1 ---------------------
                m1 = {}
                for s in SIDES:
                    tm = scp.tile([128, MLOC], BF16, tag=f"tm{s}")
                    nc.vector.tensor_scalar(tm[:], mxw[s][:, 0:MLOC],
                                            gcol[s][:, 0:1], None, ALU.mult)
                    m1t = mxp.tile([128, MLOC], BF16, tag=f"m1{s}")
                    nc.vector.scalar_tensor_tensor(
                        m1t[:], mxw[s][:, MLOC:2 * MLOC], gcol[s][:, 1:2],
                        tm[:], ALU.mult, ALU.add)
                    m1[s] = m1t

                # ---- HMN W2 + maxout, logits, local argmax per side ----
                agin = amp.tile([1, 8], F32, tag="agin")
                if not last:
                    nc.gpsimd.memset(agin[:], 0.0)
                for si_, s in enumerate(SIDES):
                    # W2; b2 folded into the scalar drains
                    w2ball = scp.tile([128, P * MLOC], BF16, tag="w2ball",
                                      name=f"w2ball{t}{si_}")
                    for pt in range(P):
                        ps = psw.tile([128, MLOC], F32, tag="w2ps")
                        nc.tensor.matmul(
                            ps[:], sb[f"w2T_{s}"][:, pt * 128:(pt + 1) * 128],
                            m1[s][:], start=True, stop=True)
                        nc.scalar.activation(
                            w2ball[:, pt * MLOC:(pt + 1) * MLOC], ps[:],
                            ACTF.Identity,
                            bias=sb[f"b2C_{s}"][:, pt:pt + 1])
                    qt1 = scp.tile([128, 4 * MLOC], BF16, tag="qt1",
                                   name=f"qt1{t}{si_}")
                    nc.vector.tensor_tensor(qt1[:], w2ball[:, 0:4 * MLOC],
                                            w2ball[:, 4 * MLOC:8 * MLOC],
                                            ALU.max)
                    qt2 = scp.tile([128, 2 * MLOC], BF16, tag="qt2",
                                   name=f"qt2{t}{si_}")
                    nc.vector.tensor_tensor(qt2[:], qt1[:, 0:2 * MLOC],
                                            qt1[:, 2 * MLOC:4 * MLOC],
                                            ALU.max)
                    m2 = mxp.tile([128, MLOC], BF16, tag=f"m2{s}")
                    nc.vector.tensor_tensor(m2[:], qt2[:, 0:MLOC],
                                            qt2[:, MLOC:2 * MLOC], ALU.max)

                    # logits: pool dim on the free axis; b3 folded via PE
                    l_ps = pss.tile([128, 4 * P], F32, tag="pss")
                    for mt in range(4):
                        lsl = l_ps[:, mt * P:(mt + 1) * P]
                        nc.tensor.matmul(lsl, onesF, sb[f"b3R_{s}"][:],
                                         start=True, stop=False)
                        nc.tensor.matmul(
                            lsl, m1[s][:, mt * 128:(mt + 1) * 128],
                            sb[f"w3T_{s}"][:, 0:P], start=False, stop=False)
                        nc.tensor.matmul(
                            lsl, m2[:, mt * 128:(mt + 1) * 128],
                            sb[f"w3T_{s}"][:, P:2 * P], start=False, stop=True)
                    # max over pool pp (innermost, stride 1): [128, 4]
                    lgc = lgp.tile([128, 4], F32, tag="lgc")
                    nc.vector.tensor_reduce(
                        lgc[:],
                        l_ps[:].rearrange("p (mt pp) -> p mt pp", pp=P),
                        axis=X, op=ALU.max)
                    nc.sync.dma_start(out_d[s][t:t + 1, :], lgc[:])

                    # ---- local (max, enc=M-idx) over [128, 4] ----------
                    if not last:
                        cmax = amp.tile([128, 1], F32, tag="cmax")
                        nc.vector.tensor_reduce(cmax[:], lgc[:], axis=X,
                                                op=ALU.max)
                        rmax_ps = pss.tile([1, 128], F32, tag="pss")
                        nc.tensor.matmul(rmax_ps[:], cmax[:], sb["ident"][:],
                                         start=True, stop=True)
                        nc.vector.tensor_reduce(
                            agin[0:1, 2 * si_:2 * si_ + 1], rmax_ps[:],
                            axis=X, op=ALU.max)
                        mb_ps = pss.tile([128, 1], F32, tag="pss")
                        nc.tensor.matmul(mb_ps[:], onesF,
                                         agin[0:1, 2 * si_:2 * si_ + 1],
                                         start=True, stop=True)
                        lmaxb = amp.tile([128, 1], F32, tag="lmaxb")
                        nc.vector.tensor_copy(lmaxb[:], mb_ps[:])
                        enc = amp.tile([128, 4], F32, tag="enc")
                        nc.vector.scalar_tensor_tensor(
                            enc[:], lgc[:], lmaxb[:, 0:1], sb["iotaM"][:],
                            ALU.is_ge, ALU.mult)
                        cenc = amp.tile([128, 1], F32, tag="cenc")
                        nc.vector.tensor_reduce(cenc[:], enc[:], axis=X,
                                                op=ALU.max)
                        renc_ps = pss.tile([1, 128], F32, tag="pss")
                        nc.tensor.matmul(renc_ps[:], cenc[:], sb["ident"][:],
                                         start=True, stop=True)
                        nc.vector.tensor_reduce(
                            agin[0:1, 2 * si_ + 1:2 * si_ + 2], renc_ps[:],
                            axis=X, op=ALU.max)

                # ---- AllGather of (max, enc) pairs; global argmax ------
                if not last:
                    ag_in = dramp.tile([1, 8], F32, tag="ag_in")
                    ag_out = dramp.tile([8, 8], F32, tag="ag_out")
                    nc.sync.dma_start(ag_in[:], agin[:])
                    nc.gpsimd.collective_compute(
                        "AllGather", ALU.bypass, replica_groups=rg,
                        ins=[ag_in.opt()], outs=[ag_out.opt()])
                    # agb cols: j*8 + rank, j in (max_s, enc_s, max_e, enc_e)
                    agb = amp.tile([1, 32], F32, tag="agb")
                    nc.sync.dma_start(
                        agb[:].rearrange("a (j r) -> a j r", r=8),
                        ag_out[:, 0:4].transpose([1, 0]))
                    idxw = amp.tile([1, 2], F32, tag="idxw")
                    for si_ in range(2):
                        cols = agb[0:1, 16 * si_:16 * si_ + 8]
                        encs = agb[0:1, 16 * si_ + 8:16 * si_ + 16]
                        gmax = amp.tile([1, 1], F32, tag=f"gmax{si_}")
                        nc.vector.tensor_reduce(gmax[:], cols, axis=X,
                                                op=ALU.max)
                        gsel = amp.tile([1, 8], F32, tag=f"gsel{si_}")
                        nc.vector.scalar_tensor_tensor(
                            gsel[:], cols, gmax[0:1, 0:1], encs,
                            ALU.is_ge, ALU.mult)
                        genc = amp.tile([1, 1], F32, tag=f"genc{si_}")
                        nc.vector.tensor_reduce(genc[:], gsel[:], axis=X,
                                                op=ALU.max)
                        # idx = M - enc
                        nc.vector.tensor_scalar(
                            idxw[0:1, si_:si_ + 1], genc[:], -1.0, float(M),
                            ALU.mult, ALU.add)
                    idx2i = amp.tile([1, 2], I32, tag="idx2i")
                    nc.vector.tensor_copy(idx2i[:], idxw[:])
                    si_v = nc.values_load(idx2i[0:1, 0:1],
                                          engines=(ET.SP, ET.Activation),
                                          min_val=0, max_val=M - 1,
                                          skip_runtime_bounds_check=True)
                    ei_v = nc.values_load(idx2i[0:1, 1:2],
                                          engines=(ET.SP, ET.Activation),
                                          min_val=0, max_val=M - 1,
                                          skip_runtime_bounds_check=True)
                    us_cols = uTj[:, :, bass.ds(si_v, 1)]
                    ue_cols = uTj[:, :, bass.ds(ei_v, 1)]

                    # pre-accumulate next step's rg bias while AG settles
                    rg_next = {}
                    for s in SIDES:
                        rg_next[s] = psr.tile([1, 130], F32, tag="rg",
                                              name=f"rg{t + 1}{s}")
                        nc.tensor.matmul(rg_next[s][:], oneB,
                                         sb[f"brgR_{s}"][:],
                                         start=True, stop=False)
                    rg_ps = rg_next

    nc.compile()
    return nc


def _pack_inputs(full):
    """Split/transform full inputs into 8 per-core input maps."""
    U = np.ascontiguousarray(np.asarray(full["U"], np.float32)[0])  # (M, 2D)
    d = D
    bf = ml_dtypes.bfloat16
    common = {}
    # uTdB: [p, j*M + m] = U[m, j*128 + p]
    uTd = np.empty((128, 2 * M), np.float32)
    for j in range(2):
        uTd[:, j * M:(j + 1) * M] = U[:, j * 128:(j + 1) * 128].T
    common["uTdB"] = uTd.astype(bf)
    perm = np.concatenate([np.arange(0, 256), np.arange(384, 512),
                           np.arange(256, 384)])     # [i, f, o, g]
    Wih = np.asarray(full["lstm_Wih"], np.float32)[perm]    # (512, 512)
    WihT = Wih.T                                      # [x, g]
    wihT = np.empty((128, 4 * 512), np.float32)
    for kx in range(4):
        wihT[:, kx * 512:(kx + 1) * 512] = WihT[kx * 128:(kx + 1) * 128, :]
    common["wihT"] = wihT.astype(bf)
    common["whhT"] = np.ascontiguousarray(
        np.asarray(full["lstm_Whh"], np.float32)[perm].T).astype(bf)
    common["bihhR"] = ((np.asarray(full["lstm_bih"], np.float32)
                        + np.asarray(full["lstm_bhh"], np.float32))[perm]
                       [None, :]).astype(bf)
    common["onesB"] = np.ones((1, 512), bf)
    common["onesF"] = np.ones((1, 128), np.float32)
    common["ident"] = np.eye(128, dtype=np.float32)
    common["zeroB"] = np.zeros((128, 1), bf)

    for s in SIDES:
        We = np.asarray(full[f"We_{s}"], np.float32)      # (K, P*D, 3D)
        be = np.asarray(full[f"be_{s}"], np.float32)      # (K, P*D)
        weT = np.empty((128, K * 2 * P * 128), np.float32)
        weR = np.empty((128, K * P * 128), np.float32)
        beC = np.empty((128, K * P), np.float32)
        for e in range(K):
            for kf in range(2):
                for pt in range(P):
                    col = ((e * 2 + kf) * P + pt) * 128
                    # lhsT[f, ec] = We[e, pt*128+ec, kf*128+f]
                    weT[:, col:col + 128] = We[e, pt * 128:(pt + 1) * 128,
                                               kf * 128:(kf + 1) * 128].T
            for pt in range(P):
                col = (pt * K + e) * 128
                weR[:, col:col + 128] = We[e, pt * 128:(pt + 1) * 128,
                                           2 * d:3 * d].T
                beC[:, pt * K + e] = be[e, pt * 128:(pt + 1) * 128]
        common[f"weT_{s}"] = weT
        common[f"weR_{s}"] = weR.astype(bf)
        common[f"beC_{s}"] = beC

        W2 = np.asarray(full[f"W2_{s}"], np.float32)      # (P*D, D)
        w2T = np.empty((128, P * 128), np.float32)
        b2C = np.empty((128, P), np.float32)
        b2 = np.asarray(full[f"b2_{s}"], np.float32)
        for pt in range(P):
            w2T[:, pt * 128:(pt + 1) * 128] = W2[pt * 128:(pt + 1) * 128, :].T
            b2C[:, pt] = b2[pt * 128:(pt + 1) * 128]
        common[f"w2T_{s}"] = w2T.astype(bf)
        common[f"b2C_{s}"] = b2C

        W3 = np.asarray(full[f"W3_{s}"], np.float32)      # (P, 2D)
        w3T = np.empty((128, 2 * P), np.float32)
        for kf in range(2):
            w3T[:, kf * P:(kf + 1) * P] = W3[:, kf * 128:(kf + 1) * 128].T
        common[f"w3T_{s}"] = w3T.astype(bf)
        common[f"b3R_{s}"] = np.asarray(full[f"b3_{s}"],
                                        np.float32)[None, :].copy()

        WrT = np.asarray(full[f"Wr_{s}"], np.float32).T   # [ctx, i]
        WgT = np.asarray(full[f"Wg_{s}"], np.float32).T   # [ctx, j]
        wrgT = np.empty((128, 5 * 130), np.float32)
        for kc in range(5):
            wrgT[:, kc * 130:kc * 130 + 128] = WrT[kc * 128:(kc + 1) * 128, :]
            wrgT[:, kc * 130 + 128:(kc + 1) * 130] = \
                WgT[kc * 128:(kc + 1) * 128, :]
        common[f"wrgT_{s}"] = wrgT.astype(bf)
        common[f"brgR_{s}"] = np.concatenate(
            [np.asarray(full[f"br_{s}"], np.float32),
             np.asarray(full[f"bg_{s}"], np.float32)])[None, :].astype(bf)

    in_maps = []
    for c in range(NCORES):
        m = dict(common)
        io = np.empty((128, 4), np.float32)
        for mt in range(4):
            io[:, mt] = M - (c * MLOC + mt * 128 + np.arange(128))
        m["iotaM"] = io
        ulocT = np.empty((128, 2 * MLOC), np.float32)
        for j in range(2):
            ulocT[:, j * MLOC:(j + 1) * MLOC] = \
                U[c * MLOC:(c + 1) * MLOC, j * 128:(j + 1) * 128].T
        m["ulocT"] = ulocT
        in_maps.append(m)
    return in_maps


def kernel(**inputs):
    if "nc" not in _CACHE:
        _CACHE["nc"] = _build()
    nc = _CACHE["nc"]
    in_maps = _pack_inputs(inputs)
    res = bass_utils.run_bass_kernel_spmd(
        nc, in_maps, core_ids=list(range(NCORES)))
    starts = np.empty((1, STEPS, M), np.float32)
    ends = np.empty((1, STEPS, M), np.float32)
    for c in range(NCORES):
        for dst, key in ((starts, "out_s"), (ends, "out_e")):
            raw = res.results[c][key]                       # [4, 512] (p,mt)
            dst[0, :, c * MLOC:(c + 1) * MLOC] = (
                raw.reshape(STEPS, 128, 4).transpose(0, 2, 1)
                .reshape(STEPS, MLOC))
    return starts, ends


# revision 9
# speedup vs baseline: 1.3318x; 1.0838x over previous
"""Trainium2 Bass kernel for nn_Decoder_81063212745440.

Pointer-network-style decoder: 4 sequential decode steps over a 4096-token
document. Each step: LSTM cell -> per-side (start/end) expert mixture with
maxout + HMN head -> per-position logits -> argmax feeds the next step.

Distribution: document dim m=4096 sharded across 8 cores (512 rows each).
Controller state is replicated. Per step each core computes its 512 local
logits; an AllGather of per-core (max, M-idx) pairs lets every core compute
the global argmax; the selected U rows are fetched by dynamic-offset DMA
from a DRAM copy of U.T.

Key structure (v3):
- z = U_loc @ We[:, :2d].T + be is step-invariant, precomputed once (f32r).
  Per-step work is the rank-1 v = We[:, 2d:] @ r bias + maxout + HMN.
- zUb blocks are laid out (pt, e)-interleaved so the two experts' maxout
  trees coalesce into one 3-level tree of wide TTs per side (measured DVE
  modes: TS bf16 ~400ns, TT bf16 2x; fused STT runs 1x -- not used).
- All per-step matvecs (LSTM, r/gate) run in bf16; W2 drains fold b2 on
  the scalar engine; b3 folds into the logits PSUM accumulation via tiny
  rank-1 matmuls.
- Warm-up AllGather is the first gpsimd instruction and has no consumers,
  so the NEFF-level collective barrier overlaps the preamble.
- argmax encoding: enc = is_ge(logit, max) * (M - idx); max enc over all
  positions/cores = first global argmax.
"""

import ml_dtypes
import numpy as np

import concourse.bacc as bacc
import concourse.bass as bass
import concourse.mybir as mybir
import concourse.tile as tile
from concourse import bass_utils

D = 128          # hidden dim d
P = 8            # maxout pool width
K = 2            # experts
STEPS = 4
M = 4096         # document length
NCORES = 8
MLOC = M // NCORES   # 512 rows per core
F32 = mybir.dt.float32
BF16 = mybir.dt.bfloat16
F32R = mybir.dt.float32r
I32 = mybir.dt.int32
X = mybir.AxisListType.X
ALU = mybir.AluOpType
ACTF = mybir.ActivationFunctionType
ET = mybir.EngineType
SIDES = ("s", "e")

_CACHE = {}


def _build():
    """Build the SPMD Bass program (identical on all cores; data differs)."""
    nc = bacc.Bacc("TRN2", target_bir_lowering=False, debug=False,
                   num_devices=NCORES)

    # ---- I/O declarations ----------------------------------------------
    inp = {}

    def din(name, shape, dt=F32):
        inp[name] = nc.dram_tensor(name, list(shape), dt, kind="ExternalInput")
        return inp[name]

    din("uTdB", (128, 2 * M), BF16)    # U.T packed: [p, j*M+m] = U[m, j*128+p]
    din("ulocT", (128, 2 * MLOC), F32R)  # per-core U slice (precompute)
    din("wihT", (128, 4 * 512), BF16)  # Wih.T k-tiles (gate order i,f,o,g)
    din("whhT", (128, 512), BF16)      # Whh.T
    din("bihhR", (1, 512), BF16)       # bih + bhh row
    din("iotaM", (128, 4))             # enc idx: [p,mt] = M - (c*512+mt*128+p)
    din("onesB", (1, 512), BF16)       # ones row (bf16 matmul helper)
    din("onesF", (1, 128))             # ones row (f32 bcast helper)
    din("ident", (128, 128))           # identity (PE partition transpose)
    din("zeroB", (128, 1), BF16)       # zero column (initial h)
    for s in SIDES:
        din(f"weT_{s}", (128, K * 2 * P * 128), F32R)
        din(f"weR_{s}", (128, K * P * 128), BF16)  # v matvec lhsT, (pt,e) order
        din(f"beC_{s}", (128, K * P))             # be cols, col = pt*K+e
        din(f"w2T_{s}", (128, P * 128), BF16)     # W2.T lhsT tiles
        din(f"b2C_{s}", (128, P))                 # b2 cols per ptile
        din(f"w3T_{s}", (128, 2 * P), BF16)       # W3.T k-tiles
        din(f"b3F_{s}", (128, 4 * P))             # b3 bcast: [p, mt*8+pp]=b3[pp]
        din(f"wrgT_{s}", (128, 5 * 130), BF16)    # [Wr | Wg].T rhs k-tiles
        din(f"brgR_{s}", (1, 130), BF16)          # [br | bg] row

    out_d = {s: nc.dram_tensor(f"out_{s}", [STEPS, MLOC], F32,
                               kind="ExternalOutput") for s in SIDES}

    rg = [list(range(NCORES))]
    uTj = inp["uTdB"].rearrange("p (j c) -> p j c", j=2)

    with (
        tile.TileContext(nc) as tc,
        tc.tile_pool(name="consts", bufs=1) as constp,
        tc.tile_pool(name="dramw", bufs=1, space="DRAM") as dramw,
    ):
        # ---- warm-up AllGather: FIRST gpsimd instruction, no consumers.
        wag_in = dramw.tile([1, 8], F32, tag="wag_in")
        wag_out = dramw.tile([8, 8], F32, tag="wag_out")
        nc.gpsimd.collective_compute(
            "AllGather", ALU.bypass, replica_groups=rg,
            ins=[wag_in.opt()], outs=[wag_out.opt()])

        # ---- persistent SBUF constants ---------------------------------
        sb = {}
        _dmai = [0]

        def _ldconst(key):
            t = constp.tile(list(inp[key].shape), inp[key].dtype,
                            tag=f"sb_{key}", name=f"sb_{key}")
            eng = (nc.sync, nc.scalar)[_dmai[0] % 2]
            _dmai[0] += 1
            eng.dma_start(t[:], inp[key][:])
            sb[key] = t

        # zUb[side]: precomputed U_loc @ WeU.T + be.
        # col block (pt*K + e)*MLOC holds tile [128(d), 512(m)].
        zUb = {s: constp.tile([128, K * P * MLOC], BF16, tag=f"zUb_{s}",
                              name=f"zUb_{s}")
               for s in SIDES}

        # ---- precompute: the step-invariant expert GEMM ----------------
        with (
            tc.tile_pool(name="prew", bufs=2) as prew,
            tc.tile_pool(name="prepsum", bufs=4, space="PSUM") as prepsum,
        ):
            uloc = prew.tile([128, 2 * MLOC], F32R, tag="ulocT")
            nc.sync.dma_start(uloc[:], inp["ulocT"][:])
            wets = {}
            for i, s in enumerate(SIDES):
                wets[s] = prew.tile([128, K * 2 * P * 128], F32R, tag="weT",
                                    name=f"weT_{s}")
                (nc.sync if i == 0 else nc.scalar).dma_start(
                    wets[s][:], inp[f"weT_{s}"][:])
            for name in ("beC_s", "beC_e", "wihT", "whhT", "bihhR", "iotaM",
                         "onesB", "onesF", "ident", "zeroB"):
                _ldconst(name)
            for s in SIDES:
                for name in ("weR", "w2T", "b2C", "w3T", "b3F",
                             "wrgT", "brgR"):
                    _ldconst(f"{name}_{s}")
            for s in SIDES:
                wet = wets[s]
                for e in range(K):
                    for pt in range(P):
                        ps = prepsum.tile([128, MLOC], F32, tag="zps")
                        for kf in range(2):
                            lcol = ((e * 2 + kf) * P + pt) * 128
                            nc.tensor.matmul(
                                ps[:],
                                wet[:, lcol:lcol + 128],
                                uloc[:, kf * MLOC:(kf + 1) * MLOC],
                                start=(kf == 0), stop=(kf == 1),
                            )
                        blk = (pt * K + e) * MLOC
                        dst = zUb[s][:, blk:blk + MLOC]
                        bcol = sb[f"beC_{s}"][:, pt * K + e:pt * K + e + 1]
                        if pt % 2 == 0:
                            nc.vector.tensor_scalar(dst, ps[:], bcol, None,
                                                    ALU.add)
                        else:
                            nc.scalar.activation(dst, ps[:], ACTF.Identity,
                                                 bias=bcol)

        oneB = sb["onesB"][0:1, 0:1]         # [1,1] == 1.0 (bf16)
        onesF = sb["onesF"][0:1, 0:128]      # [1,128] ones (f32)

        # ---- per-step pipeline -----------------------------------------
        with (
            tc.tile_pool(name="ctx", bufs=2) as ctxp,
            tc.tile_pool(name="hc", bufs=2) as hcp,
            tc.tile_pool(name="rows", bufs=4) as rowp,
            tc.tile_pool(name="zb", bufs=1) as zbp,        # bf16 zball
            tc.tile_pool(name="sc", bufs=2) as scp,        # bf16 scratch
            tc.tile_pool(name="mx", bufs=2) as mxp,
            tc.tile_pool(name="lg", bufs=2) as lgp,
            tc.tile_pool(name="am", bufs=3) as amp,
            tc.tile_pool(name="psg", bufs=1, space="PSUM") as psg,
            tc.tile_pool(name="psr", bufs=2, space="PSUM") as psr,
            tc.tile_pool(name="psw", bufs=2, space="PSUM") as psw,
            tc.tile_pool(name="pss", bufs=3, space="PSUM") as pss,
            tc.tile_pool(name="dramp", bufs=2, space="DRAM") as dramp,
        ):
            h_col = sb["zeroB"][:]
            c_row = None
            # static gather offsets for step 0: si=0, ei=M-1
            us_cols = uTj[:, :, 0:1]
            ue_cols = uTj[:, :, M - 1:M]

            # rg PSUM tiles for step 0, bias pre-accumulated
            rg_ps = {}
            for s in SIDES:
                rg_ps[s] = psr.tile([1, 130], F32, tag="rg", name=f"rg0{s}")
                nc.tensor.matmul(rg_ps[s][:], oneB, sb[f"brgR_{s}"][:],
                                 start=True, stop=False)

            for t in range(STEPS):
                last = t == STEPS - 1
                # ---- gather us/ue from DRAM into a ctx tile ------------
                ctx = ctxp.tile([128, 4], BF16, tag="ctx")
                nc.sync.dma_start(ctx[:, 0:2], us_cols)
                (nc.scalar if t > 0 else nc.sync).dma_start(
                    ctx[:, 2:4], ue_cols)

                # ---- LSTM cell (row layout) ----------------------------
                g_ps = psg.tile([1, 512], F32, tag="g")
                nc.tensor.matmul(g_ps[:], h_col, sb["whhT"][:],
                                 start=True, stop=False)
                nc.tensor.matmul(g_ps[:], oneB, sb["bihhR"][:],
                                 start=False, stop=False)
                for kx in range(4):
                    nc.tensor.matmul(
                        g_ps[:], ctx[:, kx:kx + 1],
                        sb["wihT"][:, kx * 512:(kx + 1) * 512],
                        start=False, stop=(kx == 3))
                    # rg matvec shares the ctx stationary (kc = 1+kx)
                    for s in SIDES:
                        kc = 1 + kx
                        nc.tensor.matmul(
                            rg_ps[s][:], ctx[:, kx:kx + 1],
                            sb[f"wrgT_{s}"][:, kc * 130:(kc + 1) * 130],
                            start=False, stop=False)

                # gates packed [i, f, o, g]: one sigmoid over 384 cols
                sig3 = rowp.tile([1, 384], F32, tag="sig3")
                tanh_g = rowp.tile([1, 128], F32, tag="tanh_g")
                nc.scalar.activation(sig3[:], g_ps[0:1, 0:384], ACTF.Sigmoid)
                nc.scalar.activation(tanh_g[:], g_ps[0:1, 384:512], ACTF.Tanh)

                ig = rowp.tile([1, 128], F32, tag="ig")
                c_new = hcp.tile([1, 128], F32, tag="c")
                nc.vector.tensor_tensor(ig[:], sig3[0:1, 0:128], tanh_g[:],
                                        ALU.mult)
                if c_row is None:
                    nc.vector.tensor_copy(c_new[:], ig[:])
                else:
                    fc = rowp.tile([1, 128], F32, tag="fc")
                    nc.vector.tensor_tensor(fc[:], sig3[0:1, 128:256], c_row,
                                            ALU.mult)
                    nc.vector.tensor_tensor(c_new[:], fc[:], ig[:], ALU.add)
                tanh_c = rowp.tile([1, 128], F32, tag="tanh_c")
                nc.scalar.activation(tanh_c[:], c_new[:], ACTF.Tanh)
                h_row = rowp.tile([1, 128], BF16, tag="h_row")
                nc.vector.tensor_tensor(h_row[:], sig3[0:1, 256:384],
                                        tanh_c[:], ALU.mult)
                c_row = c_new[:]

                # h transpose: [1,128] -> [128,1] via PE
                ht_ps = pss.tile([128, 2], F32, tag="pss")
                nc.tensor.matmul(ht_ps[:], h_row[:], sb["onesB"][0:1, 0:2],
                                 start=True, stop=True)
                h_new = hcp.tile([128, 1], BF16, tag="h_col")
                nc.vector.tensor_copy(h_new[:], ht_ps[:, 0:1])
                h_col = h_new[:]

                # finish rg with the h stationary (shared ldweights)
                for s in SIDES:
                    nc.tensor.matmul(rg_ps[s][:], h_col,
                                     sb[f"wrgT_{s}"][:, 0:130],
                                     start=False, stop=True)

                # ---- per-side r, gate, v -------------------------------
                gcol = {}
                vb = {}
                for s in SIDES:
                    r_row = rowp.tile([1, 128], BF16, tag=f"r_row{s}")
                    nc.scalar.activation(r_row[:], rg_ps[s][0:1, 0:128],
                                         ACTF.Tanh)
                    # gate (K=2): g0 = sigmoid(a0 - a1), g1 = 1 - g0
                    gg = rowp.tile([1, 2], F32, tag=f"gg{s}")
                    nc.vector.tensor_copy(gg[:], rg_ps[s][0:1, 128:130])
                    gd = rowp.tile([1, 1], F32, tag=f"gd{s}")
                    nc.vector.tensor_tensor(gd[:], gg[0:1, 0:1], gg[0:1, 1:2],
                                            ALU.subtract)
                    g01 = rowp.tile([1, 2], F32, tag=f"g01{s}")
                    nc.scalar.activation(g01[0:1, 0:1], gd[:], ACTF.Sigmoid)
                    nc.vector.tensor_scalar(g01[0:1, 1:2], g01[0:1, 0:1],
                                            -1.0, 1.0, ALU.mult, ALU.add)
                    # transpose r; broadcast gates to columns
                    rt_ps = pss.tile([128, 2], F32, tag="pss")
                    nc.tensor.matmul(rt_ps[:], r_row[:], sb["onesB"][0:1, 0:2],
                                     start=True, stop=True)
                    rc = rowp.tile([128, 1], BF16, tag=f"r_col{s}")
                    nc.vector.tensor_copy(rc[:], rt_ps[:, 0:1])
                    gb_ps = pss.tile([128, 2], F32, tag="pss")
                    nc.tensor.matmul(gb_ps[:], onesF, g01[:],
                                     start=True, stop=True)
                    gc = rowp.tile([128, 2], F32, tag=f"gcol{s}")
                    nc.vector.tensor_copy(gc[:], gb_ps[:])
                    gcol[s] = gc
                    # v matvec: 16 column matmuls, col = pt*K+e
                    v_ps = pss.tile([128, K * P], F32, tag="pss")
                    for pt in range(P):
                        for e in range(K):
                            lcol = (pt * K + e) * 128
                            nc.tensor.matmul(
                                v_ps[:, pt * K + e:pt * K + e + 1],
                                sb[f"weR_{s}"][:, lcol:lcol + 128],
                                rc[:], start=True, stop=True)
                    vbt = rowp.tile([128, K * P], F32, tag=f"vb{s}")
                    nc.vector.tensor_copy(vbt[:], v_ps[:])
                    vb[s] = vbt

                # ---- expert bias + coalesced maxout tree ---------------
                mxw = {}
                for s in SIDES:
                    zball = zbp.tile([128, K * P * MLOC], BF16, tag=f"zb{s}",
                                     name=f"zb{t}{s}")
                    for b in range(K * P):
                        dst = zball[:, b * MLOC:(b + 1) * MLOC]
                        zsrc = zUb[s][:, b * MLOC:(b + 1) * MLOC]
                        vcol = vb[s][:, b:b + 1]
                        if b < 4:
                            nc.scalar.activation(dst, zsrc, ACTF.Identity,
                                                 bias=vcol)
                        else:
                            nc.vector.tensor_scalar(dst, zsrc, vcol, None,
                                                    ALU.add)
                    ht1 = scp.tile([128, 8 * MLOC], BF16, tag="ht1",
                                   name=f"ht1{t}{s}")
                    nc.vector.tensor_tensor(ht1[:], zball[:, 0:8 * MLOC],
                                            zball[:, 8 * MLOC:16 * MLOC],
                                            ALU.max)
                    ht2 = scp.tile([128, 4 * MLOC], BF16, tag="ht2",
                                   name=f"ht2{t}{s}")
                    nc.vector.tensor_tensor(ht2[:], ht1[:, 0:4 * MLOC],
                                            ht1[:, 4 * MLOC:8 * MLOC],
                                            ALU.max)
                    mxe = mxp.tile([128, K * MLOC], BF16, tag=f"mx{s}")
                    nc.vector.tensor_tensor(mxe[:], ht2[:, 0:2 * MLOC],
                                            ht2[:, 2 * MLOC:4 * MLOC],
                                            ALU.max)
                    mxw[s] = mxe

                # ---- mixture: m1 = g0*mx0 + g1*mx1 ---------------------
                m1 = {}
                for s in SIDES:
                    tm = scp.tile([128, MLOC], BF16, tag=f"tm{s}")
                    nc.vector.tensor_scalar(tm[:], mxw[s][:, 0:MLOC],
                                            gcol[s][:, 0:1], None, ALU.mult)
                    m1t = mxp.tile([128, MLOC], BF16, tag=f"m1{s}")
                    nc.vector.scalar_tensor_tensor(
                        m1t[:], mxw[s][:, MLOC:2 * MLOC], gcol[s][:, 1:2],
                        tm[:], ALU.mult, ALU.add)
                    m1[s] = m1t

                # ---- HMN W2 + maxout, logits, local argmax per side ----
                agin = amp.tile([1, 8], F32, tag="agin")
                if not last:
                    nc.gpsimd.memset(agin[:], 0.0)
                # W2 GEMMs + scalar drains for both sides first, so
                # side-e's matmuls start the moment m1_e is ready.
                w2ball = {}
                for si_, s in enumerate(SIDES):
                    w2ball[s] = scp.tile([128, P * MLOC], BF16, tag="w2ball",
                                         name=f"w2ball{t}{si_}")
                    for pt in range(P):
                        ps = psw.tile([128, MLOC], F32, tag="w2ps")
                        nc.tensor.matmul(
                            ps[:], sb[f"w2T_{s}"][:, pt * 128:(pt + 1) * 128],
                            m1[s][:], start=True, stop=True)
                        nc.scalar.activation(
                            w2ball[s][:, pt * MLOC:(pt + 1) * MLOC], ps[:],
                            ACTF.Identity,
                            bias=sb[f"b2C_{s}"][:, pt:pt + 1])
                for si_, s in enumerate(SIDES):
                    qt1 = scp.tile([128, 4 * MLOC], BF16, tag="qt1",
                                   name=f"qt1{t}{si_}")
                    nc.vector.tensor_tensor(qt1[:], w2ball[s][:, 0:4 * MLOC],
                                            w2ball[s][:, 4 * MLOC:8 * MLOC],
                                            ALU.max)
                    qt2 = scp.tile([128, 2 * MLOC], BF16, tag="qt2",
                                   name=f"qt2{t}{si_}")
                    nc.vector.tensor_tensor(qt2[:], qt1[:, 0:2 * MLOC],
                                            qt1[:, 2 * MLOC:4 * MLOC],
                                            ALU.max)
                    m2 = mxp.tile([128, MLOC], BF16, tag=f"m2{s}")
                    nc.vector.tensor_tensor(m2[:], qt2[:, 0:MLOC],
                                            qt2[:, MLOC:2 * MLOC], ALU.max)

                    # logits: pool dim on the free axis
                    l_ps = pss.tile([128, 4 * P], F32, tag="pss")
                    for mt in range(4):
                        lsl = l_ps[:, mt * P:(mt + 1) * P]
                        nc.tensor.matmul(
                            lsl, m1[s][:, mt * 128:(mt + 1) * 128],
                            sb[f"w3T_{s}"][:, 0:P], start=True, stop=False)
                        nc.tensor.matmul(
                            lsl, m2[:, mt * 128:(mt + 1) * 128],
                            sb[f"w3T_{s}"][:, P:2 * P], start=False, stop=True)
                    lgb = lgp.tile([128, 4 * P], F32, tag="lgb")
                    nc.vector.tensor_tensor(lgb[:], l_ps[:], sb[f"b3F_{s}"][:],
                                            ALU.add)
                    # max over pool pp (innermost, stride 1): [128, 4]
                    lgc = lgp.tile([128, 4], F32, tag="lgc")
                    nc.vector.tensor_reduce(
                        lgc[:],
                        lgb[:].rearrange("p (mt pp) -> p mt pp", pp=P),
                        axis=X, op=ALU.max)
                    nc.sync.dma_start(out_d[s][t:t + 1, :], lgc[:])

                    # ---- local (max, enc=M-idx) over [128, 4] ----------
                    if not last:
                        cmax = amp.tile([128, 1], F32, tag="cmax")
                        nc.vector.tensor_reduce(cmax[:], lgc[:], axis=X,
                                                op=ALU.max)
                        rmax_ps = pss.tile([1, 128], F32, tag="pss")
                        nc.tensor.matmul(rmax_ps[:], cmax[:], sb["ident"][:],
                                         start=True, stop=True)
                        nc.vector.tensor_reduce(
                            agin[0:1, 2 * si_:2 * si_ + 1], rmax_ps[:],
                            axis=X, op=ALU.max)
                        mb_ps = pss.tile([128, 1], F32, tag="pss")
                        nc.tensor.matmul(mb_ps[:], onesF,
                                         agin[0:1, 2 * si_:2 * si_ + 1],
                                         start=True, stop=True)
                        lmaxb = amp.tile([128, 1], F32, tag="lmaxb")
                        nc.vector.tensor_copy(lmaxb[:], mb_ps[:])
                        enc = amp.tile([128, 4], F32, tag="enc")
                        nc.vector.scalar_tensor_tensor(
                            enc[:], lgc[:], lmaxb[:, 0:1], sb["iotaM"][:],
                            ALU.is_ge, ALU.mult)
                        cenc = amp.tile([128, 1], F32, tag="cenc")
                        nc.vector.tensor_reduce(cenc[:], enc[:], axis=X,
                                                op=ALU.max)
                        renc_ps = pss.tile([1, 128], F32, tag="pss")
                        nc.tensor.matmul(renc_ps[:], cenc[:], sb["ident"][:],
                                         start=True, stop=True)
                        nc.vector.tensor_reduce(
                            agin[0:1, 2 * si_ + 1:2 * si_ + 2], renc_ps[:],
                            axis=X, op=ALU.max)

                # ---- AllGather of (max, enc) pairs; global argmax ------
                if not last:
                    ag_in = dramp.tile([1, 8], F32, tag="ag_in")
                    ag_out = dramp.tile([8, 8], F32, tag="ag_out")
                    nc.sync.dma_start(ag_in[:], agin[:])
                    nc.gpsimd.collective_compute(
                        "AllGather", ALU.bypass, replica_groups=rg,
                        ins=[ag_in.opt()], outs=[ag_out.opt()])
                    # agb cols: j*8 + rank, j in (max_s, enc_s, max_e, enc_e)
                    agb = amp.tile([1, 32], F32, tag="agb")
                    nc.sync.dma_start(
                        agb[:].rearrange("a (j r) -> a j r", r=8),
                        ag_out[:, 0:4].transpose([1, 0]))
                    idxw = amp.tile([1, 2], F32, tag="idxw")
                    for si_ in range(2):
                        cols = agb[0:1, 16 * si_:16 * si_ + 8]
                        encs = agb[0:1, 16 * si_ + 8:16 * si_ + 16]
                        gmax = amp.tile([1, 1], F32, tag=f"gmax{si_}")
                        nc.vector.tensor_reduce(gmax[:], cols, axis=X,
                                                op=ALU.max)
                        gsel = amp.tile([1, 8], F32, tag=f"gsel{si_}")
                        nc.vector.scalar_tensor_tensor(
                            gsel[:], cols, gmax[0:1, 0:1], encs,
                            ALU.is_ge, ALU.mult)
                        genc = amp.tile([1, 1], F32, tag=f"genc{si_}")
                        nc.vector.tensor_reduce(genc[:], gsel[:], axis=X,
                                                op=ALU.max)
                        # idx = M - enc
                        nc.vector.tensor_scalar(
                            idxw[0:1, si_:si_ + 1], genc[:], -1.0, float(M),
                            ALU.mult, ALU.add)
                    idx2i = amp.tile([1, 2], I32, tag="idx2i")
                    nc.vector.tensor_copy(idx2i[:], idxw[:])
                    si_v = nc.values_load(idx2i[0:1, 0:1],
                                          engines=(ET.SP, ET.Activation),
                                          min_val=0, max_val=M - 1,
                                          skip_runtime_bounds_check=True)
                    ei_v = nc.values_load(idx2i[0:1, 1:2],
                                          engines=(ET.SP, ET.Activation),
                                          min_val=0, max_val=M - 1,
                                          skip_runtime_bounds_check=True)
                    us_cols = uTj[:, :, bass.ds(si_v, 1)]
                    ue_cols = uTj[:, :, bass.ds(ei_v, 1)]

                    # pre-accumulate next step's rg bias while AG settles
                    rg_next = {}
                    for s in SIDES:
                        rg_next[s] = psr.tile([1, 130], F32, tag="rg",
                                              name=f"rg{t + 1}{s}")
                        nc.tensor.matmul(rg_next[s][:], oneB,
                                         sb[f"brgR_{s}"][:],
                                         start=True, stop=False)
                    rg_ps = rg_next

    nc.compile()
    return nc


def _pack_inputs(full):
    """Split/transform full inputs into 8 per-core input maps."""
    U = np.ascontiguousarray(np.asarray(full["U"], np.float32)[0])  # (M, 2D)
    d = D
    bf = ml_dtypes.bfloat16
    common = {}
    # uTdB: [p, j*M + m] = U[m, j*128 + p]
    uTd = np.empty((128, 2 * M), np.float32)
    for j in range(2):
        uTd[:, j * M:(j + 1) * M] = U[:, j * 128:(j + 1) * 128].T
    common["uTdB"] = uTd.astype(bf)
    perm = np.concatenate([np.arange(0, 256), np.arange(384, 512),
                           np.arange(256, 384)])     # [i, f, o, g]
    Wih = np.asarray(full["lstm_Wih"], np.float32)[perm]    # (512, 512)
    WihT = Wih.T                                      # [x, g]
    wihT = np.empty((128, 4 * 512), np.float32)
    for kx in range(4):
        wihT[:, kx * 512:(kx + 1) * 512] = WihT[kx * 128:(kx + 1) * 128, :]
    common["wihT"] = wihT.astype(bf)
    common["whhT"] = np.ascontiguousarray(
        np.asarray(full["lstm_Whh"], np.float32)[perm].T).astype(bf)
    common["bihhR"] = ((np.asarray(full["lstm_bih"], np.float32)
                        + np.asarray(full["lstm_bhh"], np.float32))[perm]
                       [None, :]).astype(bf)
    common["onesB"] = np.ones((1, 512), bf)
    common["onesF"] = np.ones((1, 128), np.float32)
    common["ident"] = np.eye(128, dtype=np.float32)
    common["zeroB"] = np.zeros((128, 1), bf)

    for s in SIDES:
        We = np.asarray(full[f"We_{s}"], np.float32)      # (K, P*D, 3D)
        be = np.asarray(full[f"be_{s}"], np.float32)      # (K, P*D)
        weT = np.empty((128, K * 2 * P * 128), np.float32)
        weR = np.empty((128, K * P * 128), np.float32)
        beC = np.empty((128, K * P), np.float32)
        for e in range(K):
            for kf in range(2):
                for pt in range(P):
                    col = ((e * 2 + kf) * P + pt) * 128
                    # lhsT[f, ec] = We[e, pt*128+ec, kf*128+f]
                    weT[:, col:col + 128] = We[e, pt * 128:(pt + 1) * 128,
                                               kf * 128:(kf + 1) * 128].T
            for pt in range(P):
                col = (pt * K + e) * 128
                weR[:, col:col + 128] = We[e, pt * 128:(pt + 1) * 128,
                                           2 * d:3 * d].T
                beC[:, pt * K + e] = be[e, pt * 128:(pt + 1) * 128]
        common[f"weT_{s}"] = weT
        common[f"weR_{s}"] = weR.astype(bf)
        common[f"beC_{s}"] = beC

        W2 = np.asarray(full[f"W2_{s}"], np.float32)      # (P*D, D)
        w2T = np.empty((128, P * 128), np.float32)
        b2C = np.empty((128, P), np.float32)
        b2 = np.asarray(full[f"b2_{s}"], np.float32)
        for pt in range(P):
            w2T[:, pt * 128:(pt + 1) * 128] = W2[pt * 128:(pt + 1) * 128, :].T
            b2C[:, pt] = b2[pt * 128:(pt + 1) * 128]
        common[f"w2T_{s}"] = w2T.astype(bf)
        common[f"b2C_{s}"] = b2C

        W3 = np.asarray(full[f"W3_{s}"], np.float32)      # (P, 2D)
        w3T = np.empty((128, 2 * P), np.float32)
        for kf in range(2):
            w3T[:, kf * P:(kf + 1) * P] = W3[:, kf * 128:(kf + 1) * 128].T
        common[f"w3T_{s}"] = w3T.astype(bf)
        b3 = np.asarray(full[f"b3_{s}"], np.float32)
        common[f"b3F_{s}"] = np.broadcast_to(
            np.tile(b3, 4)[None, :], (128, 4 * P)).copy()

        WrT = np.asarray(full[f"Wr_{s}"], np.float32).T   # [ctx, i]
        WgT = np.asarray(full[f"Wg_{s}"], np.float32).T   # [ctx, j]
        wrgT = np.empty((128, 5 * 130), np.float32)
        for kc in range(5):
            wrgT[:, kc * 130:kc * 130 + 128] = WrT[kc * 128:(kc + 1) * 128, :]
            wrgT[:, kc * 130 + 128:(kc + 1) * 130] = \
                WgT[kc * 128:(kc + 1) * 128, :]
        common[f"wrgT_{s}"] = wrgT.astype(bf)
        common[f"brgR_{s}"] = np.concatenate(
            [np.asarray(full[f"br_{s}"], np.float32),
             np.asarray(full[f"bg_{s}"], np.float32)])[None, :].astype(bf)

    in_maps = []
    for c in range(NCORES):
        m = dict(common)
        io = np.empty((128, 4), np.float32)
        for mt in range(4):
            io[:, mt] = M - (c * MLOC + mt * 128 + np.arange(128))
        m["iotaM"] = io
        ulocT = np.empty((128, 2 * MLOC), np.float32)
        for j in range(2):
            ulocT[:, j * MLOC:(j + 1) * MLOC] = \
                U[c * MLOC:(c + 1) * MLOC, j * 128:(j + 1) * 128].T
        m["ulocT"] = ulocT
        in_maps.append(m)
    return in_maps


def kernel(**inputs):
    if "nc" not in _CACHE:
        _CACHE["nc"] = _build()
    nc = _CACHE["nc"]
    in_maps = _pack_inputs(inputs)
    res = bass_utils.run_bass_kernel_spmd(
        nc, in_maps, core_ids=list(range(NCORES)))
    starts = np.empty((1, STEPS, M), np.float32)
    ends = np.empty((1, STEPS, M), np.float32)
    for c in range(NCORES):
        for dst, key in ((starts, "out_s"), (ends, "out_e")):
            raw = res.results[c][key]                       # [4, 512] (p,mt)
            dst[0, :, c * MLOC:(c + 1) * MLOC] = (
                raw.reshape(STEPS, 128, 4).transpose(0, 2, 1)
                .reshape(STEPS, MLOC))
    return starts, ends
